# revision 8
# baseline (speedup 1.0000x reference)
"""CascadeXML top-k cascade kernel for Trainium2 (Bass/Tile), 8-core SPMD.

Data-parallel over batch (B=64 -> 8 rows/core); each core runs the full
cascade on its rows. HW constraint: indirect DMA supports ONE offset per
partition ([P,1]), so table gathers run as per-column [128,1] calls and
index reshapes route through DRAM scratch (DRAM APs are unconstrained).

v2 vs baseline: host-packed big-descriptor weight loads issued on the
scalar HWDGE queue, level-0 GEMM pipelined per 512-col block with sigmoid
and stage-1 topk overlapped, gid recovery via per-window max_index + PE
selector combine (instead of [8,2048] scans), fused gather-dots via
scalar_tensor_tensor accum_out.
"""

import os
import sys

for _p in ("/opt/trn_rl_repo",):
    if _p not in sys.path:
        sys.path.insert(0, _p)

import numpy as np

B, EMB = 64, 768
N0, N1, NL = 2048, 16384, 131072
CS, K = 8, 50
NCORES = 8
BL = B // NCORES          # 8 rows per core
ROUNDS = (K + 7) // 8     # 7 rounds of max8 -> 56 >= 50
NSEL = ROUNDS * 8         # 56
NCAND = K * CS            # 400
QG = 128 // BL            # 16
NJ = NCAND * BL // 128    # 25 slots per partition in g-layout
NCHUNK = 5
JPC = NJ // NCHUNK        # 5
KCH0 = (2 * EMB) // 128   # 12
MCH = EMB // 128          # 6
NBLK = 4                  # 512-col blocks of N0
WPB = 4                   # 128-wide windows per block
OUTW = N0 + 2 * NCAND     # 2848

_cached = {}


def _build():
    import concourse.bacc as bacc
    import concourse.bass as bass
    import concourse.mybir as mybir
    from concourse.masks import make_identity
    from concourse.tile import TileContext

    f32 = mybir.dt.float32
    i32 = mybir.dt.int32
    u32 = mybir.dt.uint32
    AF = mybir.ActivationFunctionType
    ALU = mybir.AluOpType

    nc = bacc.Bacc(num_devices=NCORES)

    feat0P = nc.dram_tensor("feat0P", [128, KCH0 * BL], f32, kind="ExternalInput")
    WhP = nc.dram_tensor("WhP", [128, KCH0 * EMB], f32, kind="ExternalInput")
    C0P = nc.dram_tensor("C0P", [128, MCH * N0], f32, kind="ExternalInput")
    f1rep = nc.dram_tensor("f1rep", [128, EMB], f32, kind="ExternalInput")
    f2rep = nc.dram_tensor("f2rep", [128, EMB], f32, kind="ExternalInput")
    C1 = nc.dram_tensor("C1", [N1, EMB], f32, kind="ExternalInput")
    C2 = nc.dram_tensor("C2", [NL, EMB], f32, kind="ExternalInput")
    clusters0 = nc.dram_tensor("clusters0", [N0, CS], i32, kind="ExternalInput")
    clusters1 = nc.dram_tensor("clusters1", [N1, CS], i32, kind="ExternalInput")
    Gsel = nc.dram_tensor("Gsel", [128, 2 * BL], f32, kind="ExternalInput")
    Gbc = nc.dram_tensor("Gbc", [BL, 128], f32, kind="ExternalInput")
    brow400 = nc.dram_tensor("brow400", [BL, 1], u32, kind="ExternalInput")
    out = nc.dram_tensor("out", [BL, OUTW], f32, kind="ExternalOutput")

    d_v56 = nc.dram_tensor("d_v56", [128, NSEL], f32)
    d_idx1 = nc.dram_tensor("d_idx1", [BL * K, 1], u32)     # [400,1]
    d_cand1 = nc.dram_tensor("d_cand1", [BL * NCAND, 1], i32)
    d_fidx = nc.dram_tensor("d_fidx", [BL * K, 1], u32)
    d_cand2 = nc.dram_tensor("d_cand2", [BL * NCAND, 1], i32)

    with TileContext(nc) as tc:
        with (
            tc.tile_pool(name="consts", bufs=1) as consts,
            tc.tile_pool(name="ev", bufs=2) as ev_pool,
            tc.tile_pool(name="work", bufs=1) as work,
            tc.tile_pool(name="ph", bufs=1, space="PSUM") as ph_pool,
            tc.tile_pool(name="pt", bufs=2, space="PSUM") as pt_pool,
            tc.tile_pool(name="pl", bufs=2, space="PSUM") as pl_pool,
            tc.tile_pool(name="pg", bufs=1, space="PSUM") as pg_pool,
        ):
            # ---- consts (sync queue; small) ----
            s_feat0P = consts.tile([128, KCH0 * BL], f32)
            nc.sync.dma_start(out=s_feat0P[:], in_=feat0P[:])
            s_ident = consts.tile([128, 128], f32)
            make_identity(nc, s_ident[:])
            s_f1rep = consts.tile([128, EMB], f32)
            nc.sync.dma_start(out=s_f1rep[:], in_=f1rep[:])
            s_f2rep = consts.tile([128, EMB], f32)
            nc.sync.dma_start(out=s_f2rep[:], in_=f2rep[:])
            s_Gsel = consts.tile([128, 2 * BL], f32)
            nc.sync.dma_start(out=s_Gsel[:], in_=Gsel[:])
            s_Gbc = consts.tile([BL, 128], f32)
            nc.sync.dma_start(out=s_Gbc[:], in_=Gbc[:])
            s_brow400 = consts.tile([BL, 1], u32)
            nc.sync.dma_start(out=s_brow400[:], in_=brow400[:])

            # ---- big weight loads on the scalar HWDGE queue ----
            s_WhP = consts.tile([128, KCH0 * EMB], f32)
            for h in range(3):
                sl = slice(4 * EMB * h, 4 * EMB * (h + 1))
                nc.sync.dma_start(out=s_WhP[:, sl], in_=WhP[:, sl])
            s_C0P = consts.tile([128, MCH * N0], f32)
            for n in range(NBLK):
                sl = slice(MCH * 512 * n, MCH * 512 * (n + 1))
                nc.sync.dma_start(out=s_C0P[:, sl], in_=C0P[:, sl])

            # ---- phase A: h0 = feat0 @ Wh.T -> [8, 768] ----
            ph0a = ph_pool.tile([BL, 512], f32)
            ph0b = ph_pool.tile([BL, 256], f32)
            for k in range(KCH0):
                lhs = s_feat0P[:, BL * k:BL * (k + 1)]
                rhs = s_WhP[:, EMB * k:EMB * (k + 1)]
                nc.tensor.matmul(ph0a[:], lhsT=lhs, rhs=rhs[:, 0:512],
                                 start=(k == 0), stop=(k == KCH0 - 1))
                nc.tensor.matmul(ph0b[:], lhsT=lhs, rhs=rhs[:, 512:768],
                                 start=(k == 0), stop=(k == KCH0 - 1))
            s_h0 = work.tile([BL, EMB], f32)
            nc.vector.tensor_copy(s_h0[:, 0:512], ph0a[:])
            nc.vector.tensor_copy(s_h0[:, 512:768], ph0b[:])

            # ---- phase B: h0T via PE transpose ----
            s_h0T = work.tile([128, MCH * BL], f32)
            for m in range(MCH):
                ptile = pt_pool.tile([128, BL], f32, tag="pt")
                nc.tensor.transpose(ptile[:], s_h0[:, 128 * m:128 * (m + 1)],
                                    s_ident[:BL, :BL])
                nc.vector.tensor_copy(s_h0T[:, BL * m:BL * (m + 1)], ptile[:])

            # ---- phase C+D: logits0 blocks -> probs0 + stage-1 topk ----
            # g-layout partition p = 32n + 4b + q_l (block-local b-major)
            s_probs0 = work.tile([BL, N0], f32)
            s_p0g = work.tile([128, 128], f32)    # pristine
            s_p0w = work.tile([128, 128], f32)    # match_replace workspace
            s_vals56 = work.tile([128, NSEL], f32)
            for n in range(NBLK):
                pl = pl_pool.tile([BL, 512], f32, tag="pl0")
                for kk in range(MCH):
                    rhs = s_C0P[:, MCH * 512 * n + 512 * kk:
                                 MCH * 512 * n + 512 * (kk + 1)]
                    nc.tensor.matmul(pl[:], lhsT=s_h0T[:, BL * kk:BL * (kk + 1)],
                                     rhs=rhs, start=(kk == 0), stop=(kk == MCH - 1))
                nc.scalar.activation(s_probs0[:, 512 * n:512 * (n + 1)], pl[:],
                                     AF.Sigmoid)
                psl = slice(32 * n, 32 * (n + 1))
                nc.sync.dma_start(
                    out=s_p0g[psl, :],
                    in_=s_probs0[:, 512 * n:512 * (n + 1)]
                        .rearrange("b (q f) -> b q f", f=128),
                )
                nc.vector.tensor_copy(s_p0w[psl, :], s_p0g[psl, :])
                for r in range(ROUNDS):
                    sl = slice(8 * r, 8 * r + 8)
                    nc.vector.max(s_vals56[psl, sl], s_p0w[psl, :])
                    nc.vector.match_replace(s_p0w[psl, :], s_vals56[psl, sl],
                                            s_p0w[psl, :], -1.0)
            nc.sync.dma_start(out=out[:, 0:N0], in_=s_probs0[:])

            # ---- stage-2 merge: 16 sorted lists -> sorted top-56 per row ----
            nc.sync.dma_start(out=d_v56[:], in_=s_vals56[:])
            s_v896 = work.tile([BL, QG * NSEL], f32)
            nc.sync.dma_start(
                out=s_v896[:].rearrange("b (n q r) -> b n q r", n=NBLK, q=WPB),
                in_=d_v56[:].rearrange("(n b q) r -> b n q r", n=NBLK, b=BL),
            )
            s_vals1 = work.tile([BL, NSEL], f32)
            for r in range(ROUNDS):
                sl = slice(8 * r, 8 * r + 8)
                nc.vector.max(s_vals1[:, sl], s_v896[:])
                nc.vector.match_replace(s_v896[:], s_vals1[:, sl], s_v896[:], -1.0)

            s_gid56 = work.tile([BL, NSEL], u32)
            if os.environ.get("KERNEL_GID_BASELINE"):
                # baseline gid: scan the full [8, 2048] row per round
                for r in range(ROUNDS):
                    sl = slice(8 * r, 8 * r + 8)
                    nc.vector.max_index(s_gid56[:, sl], s_vals1[:, sl],
                                        s_probs0[:])
            else:
                # per-window max_index + PE selector combine
                pq = pg_pool.tile([128, NSEL], f32, tag="pq")
                nc.tensor.matmul(pq[:], lhsT=s_Gbc[:], rhs=s_vals1[:],
                                 start=True, stop=True)
                s_q56 = work.tile([128, NSEL], f32)
                nc.vector.tensor_copy(s_q56[:], pq[:])
                s_posu = work.tile([128, NSEL], u32)
                s_posf = work.tile([128, NSEL], f32)
                s_pm = work.tile([128, NSEL], f32)
                s_m = work.tile([128, NSEL], f32)
                pS = pg_pool.tile([BL, 2 * NSEL], f32, tag="pS")
                for r in range(ROUNDS):
                    sl = slice(8 * r, 8 * r + 8)
                    sl2 = slice(NSEL + 8 * r, NSEL + 8 * r + 8)
                    nc.vector.max_index(s_posu[:, sl], s_q56[:, sl], s_p0g[:])
                    nc.vector.tensor_copy(s_posf[:, sl], s_posu[:, sl])
                    # unmatched -> pos = 2^32-1 as float; mask = pos < 2048
                    nc.vector.tensor_scalar(s_m[:, sl], s_posf[:, sl], 2048.0,
                                            None, op0=ALU.less_than)
                    nc.vector.tensor_mul(s_pm[:, sl], s_posf[:, sl], s_m[:, sl])
                    nc.tensor.matmul(pS[:, sl], lhsT=s_Gsel[:, 0:BL],
                                     rhs=s_pm[:, sl], start=True, stop=True)
                    nc.tensor.matmul(pS[:, sl2], lhsT=s_Gsel[:, BL:],
                                     rhs=s_m[:, sl], start=True, stop=True)
                # gid = S1 + 128*S2
                s_gidf = work.tile([BL, NSEL], f32)
                nc.vector.scalar_tensor_tensor(
                    out=s_gidf[:], in0=pS[:, NSEL:], scalar=128.0,
                    in1=pS[:, 0:NSEL], op0=ALU.mult, op1=ALU.add)
                nc.vector.tensor_copy(s_gid56[:], s_gidf[:])

            # ---- phase E: cand1 = clusters0[idx1] (DRAM-bounce reshapes) ----
            nc.sync.dma_start(out=d_idx1[:], in_=s_gid56[:, 0:K])
            s_idx1p = work.tile([100, 4], u32)
            nc.sync.dma_start(
                out=s_idx1p[:],
                in_=d_idx1[:].rearrange("(t P) one -> P (t one)", P=100),
            )
            s_c1raw = work.tile([100, 4 * CS], i32)
            for t in range(4):
                nc.gpsimd.indirect_dma_start(
                    out=s_c1raw[:, CS * t:CS * (t + 1)], out_offset=None,
                    in_=clusters0[:],
                    in_offset=bass.IndirectOffsetOnAxis(
                        ap=s_idx1p[:, t:t + 1], axis=0),
                )
            nc.sync.dma_start(
                out=d_cand1[:].rearrange("(t P m) one -> P t (m one)", P=100, m=CS),
                in_=s_c1raw[:],
            )
            s_cand1g = work.tile([128, NJ], i32)
            nc.sync.dma_start(
                out=s_cand1g[:],
                in_=d_cand1[:].rearrange("(b q j) one -> (b q) (j one)", q=QG, j=NJ),
            )

            # ---- phase F: gather C1 rows + fused dots ----
            s_logits1g = work.tile([128, NJ], f32)
            s_scr = work.tile([128, EMB], f32)
            for ch in range(NCHUNK):
                et = ev_pool.tile([128, JPC * EMB], f32, tag="ev")
                for jj in range(JPC):
                    j = JPC * ch + jj
                    nc.gpsimd.indirect_dma_start(
                        out=et[:, EMB * jj:EMB * (jj + 1)], out_offset=None,
                        in_=C1[:],
                        in_offset=bass.IndirectOffsetOnAxis(
                            ap=s_cand1g[:, j:j + 1], axis=0),
                    )
                for jj in range(JPC):
                    j = JPC * ch + jj
                    nc.vector.scalar_tensor_tensor(
                        out=s_scr[:], in0=et[:, EMB * jj:EMB * (jj + 1)],
                        scalar=1.0, in1=s_f1rep[:],
                        op0=ALU.mult, op1=ALU.mult,
                        accum_out=s_logits1g[:, j:j + 1])

            # ---- phase G: probs1, top-50, w1 ----
            s_probs1g = work.tile([128, NJ], f32)
            nc.scalar.activation(s_probs1g[:], s_logits1g[:], AF.Sigmoid)
            s_probs1b = work.tile([BL, NCAND], f32)
            nc.sync.dma_start(
                out=s_probs1b[:].rearrange("b (q j) -> b q j", j=NJ),
                in_=s_probs1g[:],
            )
            s_p1w = work.tile([BL, NCAND], f32)
            nc.vector.tensor_copy(s_p1w[:], s_probs1b[:])
            s_vals2 = work.tile([BL, NSEL], f32)
            s_pos2 = work.tile([BL, NSEL], u32)
            for r in range(ROUNDS):
                sl = slice(8 * r, 8 * r + 8)
                nc.vector.max(s_vals2[:, sl], s_p1w[:])
                nc.vector.max_index(s_pos2[:, sl], s_vals2[:, sl], s_p1w[:])
                nc.vector.match_replace(s_p1w[:], s_vals2[:, sl], s_p1w[:], -1.0)
            s_g1 = work.tile([BL, NCAND], f32)
            nc.vector.tensor_copy(
                s_g1[:].rearrange("b (k m) -> b k m", m=CS),
                s_vals1[:, 0:K].to_broadcast([BL, K, CS]),
            )
            s_w1 = work.tile([BL, NCAND], f32)
            nc.vector.tensor_mul(s_w1[:], s_probs1b[:], s_g1[:])
            nc.sync.dma_start(out=out[:, N0:N0 + NCAND], in_=s_w1[:])

            # ---- phase H: level-2 candidate ids ----
            s_fidx = work.tile([BL, K], u32)
            nc.vector.tensor_tensor(s_fidx[:], s_pos2[:, 0:K],
                                    s_brow400[:].to_broadcast([BL, K]),
                                    op=ALU.add)
            nc.sync.dma_start(out=d_fidx[:], in_=s_fidx[:])
            s_fidxp = work.tile([100, 4], u32)
            nc.sync.dma_start(
                out=s_fidxp[:],
                in_=d_fidx[:].rearrange("(t P) one -> P (t one)", P=100),
            )
            s_ind2raw = work.tile([100, 4], i32)
            for t in range(4):
                nc.gpsimd.indirect_dma_start(
                    out=s_ind2raw[:, t:t + 1], out_offset=None, in_=d_cand1[:],
                    in_offset=bass.IndirectOffsetOnAxis(
                        ap=s_fidxp[:, t:t + 1], axis=0),
                )
            s_c2raw = work.tile([100, 4 * CS], i32)
            for t in range(4):
                nc.gpsimd.indirect_dma_start(
                    out=s_c2raw[:, CS * t:CS * (t + 1)], out_offset=None,
                    in_=clusters1[:],
                    in_offset=bass.IndirectOffsetOnAxis(
                        ap=s_ind2raw[:, t:t + 1], axis=0),
                )
            nc.sync.dma_start(
                out=d_cand2[:].rearrange("(t P m) one -> P t (m one)", P=100, m=CS),
                in_=s_c2raw[:],
            )
            s_cand2g = work.tile([128, NJ], i32)
            nc.sync.dma_start(
                out=s_cand2g[:],
                in_=d_cand2[:].rearrange("(b q j) one -> (b q) (j one)", q=QG, j=NJ),
            )

            # ---- phase I: gather C2 rows + fused dots, probs2, w2 ----
            s_logits2g = work.tile([128, NJ], f32)
            for ch in range(NCHUNK):
                et = ev_pool.tile([128, JPC * EMB], f32, tag="ev")
                for jj in range(JPC):
                    j = JPC * ch + jj
                    nc.gpsimd.indirect_dma_start(
                        out=et[:, EMB * jj:EMB * (jj + 1)], out_offset=None,
                        in_=C2[:],
                        in_offset=bass.IndirectOffsetOnAxis(
                            ap=s_cand2g[:, j:j + 1], axis=0),
                    )
                for jj in range(JPC):
                    j = JPC * ch + jj
                    nc.vector.scalar_tensor_tensor(
                        out=s_scr[:], in0=et[:, EMB * jj:EMB * (jj + 1)],
                        scalar=1.0, in1=s_f2rep[:],
                        op0=ALU.mult, op1=ALU.mult,
                        accum_out=s_logits2g[:, j:j + 1])
            s_probs2g = work.tile([128, NJ], f32)
            nc.scalar.activation(s_probs2g[:], s_logits2g[:], AF.Sigmoid)
            s_mask = work.tile([128, NJ], f32)
            nc.vector.tensor_scalar(s_mask[:], s_logits2g[:], 0.0, None,
                                    op0=ALU.not_equal)
            nc.vector.tensor_mul(s_probs2g[:], s_probs2g[:], s_mask[:])
            s_probs2b = work.tile([BL, NCAND], f32)
            nc.sync.dma_start(
                out=s_probs2b[:].rearrange("b (q j) -> b q j", j=NJ),
                in_=s_probs2g[:],
            )
            s_g2 = work.tile([BL, NCAND], f32)
            nc.vector.tensor_copy(
                s_g2[:].rearrange("b (k m) -> b k m", m=CS),
                s_vals2[:, 0:K].to_broadcast([BL, K, CS]),
            )
            s_w2 = work.tile([BL, NCAND], f32)
            nc.vector.tensor_mul(s_w2[:], s_probs2b[:], s_g2[:])
            nc.sync.dma_start(out=out[:, N0 + NCAND:OUTW], in_=s_w2[:])

    nc.compile()
    return nc


def _get_nc():
    if "nc" not in _cached:
        _cached["nc"] = _build()
    return _cached["nc"]


def _make_in_maps(feat0, feat1, feat2, Wh, bh, C0, b0, C1, b1, C2, b2,
                  clusters0, clusters1):
    WhT = np.ascontiguousarray(Wh.T)            # [1536, 768]
    C0T = np.ascontiguousarray(C0.T)            # [768, 2048]
    feat0T = np.ascontiguousarray(feat0.T)      # [1536, 64]
    WhP = np.ascontiguousarray(
        WhT.reshape(KCH0, 128, EMB).transpose(1, 0, 2).reshape(128, KCH0 * EMB))
    C0P = np.ascontiguousarray(
        C0T.reshape(MCH, 128, NBLK, 512).transpose(1, 2, 0, 3)
           .reshape(128, NBLK * MCH * 512))
    brow400 = (NCAND * np.arange(BL, dtype=np.uint32)).reshape(BL, 1)
    c0 = np.ascontiguousarray(clusters0.astype(np.int32))
    c1 = np.ascontiguousarray(clusters1.astype(np.int32))
    # level-0 g-layout: p = 32n + 4b + q_l; row(p) = (p%32)//4,
    # window(p) = 4*(p//32) + p%4
    p = np.arange(128)
    rowp = (p % 32) // 4
    winp = 4 * (p // 32) + p % 4
    gb = (rowp[:, None] == np.arange(BL)[None, :]).astype(np.float32)
    gq = gb * winp[:, None].astype(np.float32)
    Gsel_np = np.ascontiguousarray(np.concatenate([gb, gq], axis=1))
    Gbc_np = np.ascontiguousarray(gb.T)
    in_maps = []
    for c in range(NCORES):
        rows = slice(BL * c, BL * (c + 1))
        f0P = np.ascontiguousarray(
            feat0T[:, rows].reshape(KCH0, 128, BL).transpose(1, 0, 2)
                  .reshape(128, KCH0 * BL))
        in_maps.append({
            "feat0P": f0P,
            "WhP": WhP,
            "C0P": C0P,
            "f1rep": np.repeat(feat1[rows], QG, axis=0),
            "f2rep": np.repeat(feat2[rows], QG, axis=0),
            "C1": C1,
            "C2": C2,
            "clusters0": c0,
            "clusters1": c1,
            "Gsel": Gsel_np,
            "Gbc": Gbc_np,
            "brow400": brow400,
        })
    return in_maps


def kernel(**inputs):
    nc = _get_nc()
    in_maps = _make_in_maps(**inputs)
    if os.environ.get("BASS_KERNEL_SIM"):
        from concourse.bass_interp import CoreSim
        outs = []
        for c in range(NCORES):
            sim = CoreSim(nc)
            for name, arr in in_maps[c].items():
                sim.tensor(name)[:] = arr
            sim.simulate()
            outs.append(np.array(sim.tensor("out")))
        return np.concatenate(outs, axis=0)
    from concourse.bass_utils import run_bass_kernel_spmd
    trace = bool(os.environ.get("BASS_KERNEL_TRACE"))
    res = run_bass_kernel_spmd(nc, in_maps, core_ids=list(range(NCORES)),
                               trace=trace)
    _cached["last_exec_ns"] = res.exec_time_ns
    _cached["last_results"] = res
    return np.concatenate([res.results[c]["out"] for c in range(NCORES)], axis=0)


if __name__ == "__main__":
    _get_nc()
    print("build+compile OK")


# revision 12
# speedup vs baseline: 1.0263x; 1.0263x over previous
"""CascadeXML top-k cascade kernel for Trainium2 (Bass/Tile), 8-core SPMD.

Data-parallel over batch (B=64 -> 8 rows/core); each core runs the full
cascade on its rows. HW constraint: indirect DMA supports ONE offset per
partition ([P,1]), so table gathers run as per-column [128,1] calls and
index reshapes route through DRAM scratch (DRAM APs are unconstrained).

v2 vs baseline: host-packed big-descriptor weight loads issued on the
scalar HWDGE queue, level-0 GEMM pipelined per 512-col block with sigmoid
and stage-1 topk overlapped, gid recovery via per-window max_index + PE
selector combine (instead of [8,2048] scans), fused gather-dots via
scalar_tensor_tensor accum_out.
"""

import os
import sys

for _p in ("/opt/trn_rl_repo",):
    if _p not in sys.path:
        sys.path.insert(0, _p)

import numpy as np

B, EMB = 64, 768
N0, N1, NL = 2048, 16384, 131072
CS, K = 8, 50
NCORES = 8
BL = B // NCORES          # 8 rows per core
ROUNDS = (K + 7) // 8     # 7 rounds of max8 -> 56 >= 50
NSEL = ROUNDS * 8         # 56
NCAND = K * CS            # 400
QG = 128 // BL            # 16
NJ = NCAND * BL // 128    # 25 slots per partition in g-layout
NCHUNK = 5
JPC = NJ // NCHUNK        # 5
KCH0 = (2 * EMB) // 128   # 12
MCH = EMB // 128          # 6
NBLK = 4                  # 512-col blocks of N0
WPB = 4                   # 128-wide windows per block
OUTW = N0 + 2 * NCAND     # 2848

_cached = {}


def _build():
    import concourse.bacc as bacc
    import concourse.bass as bass
    import concourse.mybir as mybir
    from concourse.masks import make_identity
    from concourse.tile import TileContext

    f32 = mybir.dt.float32
    i32 = mybir.dt.int32
    u32 = mybir.dt.uint32
    AF = mybir.ActivationFunctionType
    ALU = mybir.AluOpType

    nc = bacc.Bacc(num_devices=NCORES)

    feat0P = nc.dram_tensor("feat0P", [128, KCH0 * BL], f32, kind="ExternalInput")
    WhP = nc.dram_tensor("WhP", [128, KCH0 * EMB], f32, kind="ExternalInput")
    C0P = nc.dram_tensor("C0P", [128, MCH * N0], f32, kind="ExternalInput")
    f1rep = nc.dram_tensor("f1rep", [128, EMB], f32, kind="ExternalInput")
    f2rep = nc.dram_tensor("f2rep", [128, EMB], f32, kind="ExternalInput")
    C1 = nc.dram_tensor("C1", [N1, EMB], f32, kind="ExternalInput")
    C2 = nc.dram_tensor("C2", [NL, EMB], f32, kind="ExternalInput")
    clusters0 = nc.dram_tensor("clusters0", [N0, CS], i32, kind="ExternalInput")
    clusters1 = nc.dram_tensor("clusters1", [N1, CS], i32, kind="ExternalInput")
    Gsel = nc.dram_tensor("Gsel", [128, 2 * BL], f32, kind="ExternalInput")
    Gbc = nc.dram_tensor("Gbc", [BL, 128], f32, kind="ExternalInput")
    brow400 = nc.dram_tensor("brow400", [BL, 1], u32, kind="ExternalInput")
    out = nc.dram_tensor("out", [BL, OUTW], f32, kind="ExternalOutput")

    d_v56 = nc.dram_tensor("d_v56", [128, NSEL], f32)
    d_idx1 = nc.dram_tensor("d_idx1", [BL * K, 1], u32)     # [400,1]
    d_cand1 = nc.dram_tensor("d_cand1", [BL * NCAND, 1], i32)
    d_fidx = nc.dram_tensor("d_fidx", [BL * K, 1], u32)
    d_cand2 = nc.dram_tensor("d_cand2", [BL * NCAND, 1], i32)

    with TileContext(nc) as tc:
        with (
            tc.tile_pool(name="consts", bufs=1) as consts,
            tc.tile_pool(name="ev", bufs=2) as ev_pool,
            tc.tile_pool(name="work", bufs=1) as work,
            tc.tile_pool(name="ph", bufs=1, space="PSUM") as ph_pool,
            tc.tile_pool(name="pt", bufs=2, space="PSUM") as pt_pool,
            tc.tile_pool(name="pl", bufs=2, space="PSUM") as pl_pool,
            tc.tile_pool(name="pg", bufs=1, space="PSUM") as pg_pool,
        ):
            # ---- consts (sync queue; small) ----
            s_feat0P = consts.tile([128, KCH0 * BL], f32)
            nc.sync.dma_start(out=s_feat0P[:], in_=feat0P[:])
            s_ident = consts.tile([128, 128], f32)
            make_identity(nc, s_ident[:])
            s_f1rep = consts.tile([128, EMB], f32)
            nc.sync.dma_start(out=s_f1rep[:], in_=f1rep[:])
            s_f2rep = consts.tile([128, EMB], f32)
            nc.sync.dma_start(out=s_f2rep[:], in_=f2rep[:])
            s_Gsel = consts.tile([128, 2 * BL], f32)
            nc.sync.dma_start(out=s_Gsel[:], in_=Gsel[:])
            s_Gbc = consts.tile([BL, 128], f32)
            nc.sync.dma_start(out=s_Gbc[:], in_=Gbc[:])
            s_brow400 = consts.tile([BL, 1], u32)
            nc.sync.dma_start(out=s_brow400[:], in_=brow400[:])

            # ---- big weight loads on the scalar HWDGE queue ----
            s_WhP = consts.tile([128, KCH0 * EMB], f32)
            for h in range(3):
                sl = slice(4 * EMB * h, 4 * EMB * (h + 1))
                nc.sync.dma_start(out=s_WhP[:, sl], in_=WhP[:, sl])
            s_C0P = consts.tile([128, MCH * N0], f32)
            for n in range(NBLK):
                sl = slice(MCH * 512 * n, MCH * 512 * (n + 1))
                nc.sync.dma_start(out=s_C0P[:, sl], in_=C0P[:, sl])

            # ---- phase A: h0 = feat0 @ Wh.T -> [8, 768] ----
            ph0a = ph_pool.tile([BL, 512], f32)
            ph0b = ph_pool.tile([BL, 256], f32)
            for k in range(KCH0):
                lhs = s_feat0P[:, BL * k:BL * (k + 1)]
                rhs = s_WhP[:, EMB * k:EMB * (k + 1)]
                nc.tensor.matmul(ph0a[:], lhsT=lhs, rhs=rhs[:, 0:512],
                                 start=(k == 0), stop=(k == KCH0 - 1))
                nc.tensor.matmul(ph0b[:], lhsT=lhs, rhs=rhs[:, 512:768],
                                 start=(k == 0), stop=(k == KCH0 - 1))
            s_h0 = work.tile([BL, EMB], f32)
            nc.vector.tensor_copy(s_h0[:, 0:512], ph0a[:])
            nc.vector.tensor_copy(s_h0[:, 512:768], ph0b[:])

            # ---- phase B: h0T via PE transpose ----
            s_h0T = work.tile([128, MCH * BL], f32)
            for m in range(MCH):
                ptile = pt_pool.tile([128, BL], f32, tag="pt")
                nc.tensor.transpose(ptile[:], s_h0[:, 128 * m:128 * (m + 1)],
                                    s_ident[:BL, :BL])
                nc.vector.tensor_copy(s_h0T[:, BL * m:BL * (m + 1)], ptile[:])

            # ---- phase C+D: logits0 blocks -> probs0 + stage-1 topk ----
            # g-layout partition p = 32n + 4b + q_l (block-local b-major)
            s_probs0 = work.tile([BL, N0], f32)
            s_p0g = work.tile([128, 128], f32)    # pristine
            s_p0w = work.tile([128, 128], f32)    # match_replace workspace
            s_vals56 = work.tile([128, NSEL], f32)
            s_probsT = work.tile([128, 16 * BL], f32)
            for n in range(NBLK):
                # transposed GEMM: out chunk = logits0T [128 n, 8 b]
                for cl in range(4):
                    c = 4 * n + cl
                    pl = pl_pool.tile([128, BL], f32, tag="pl0")
                    for kk in range(MCH):
                        lhsT = s_C0P[:, 128 * (MCH * c + kk):
                                     128 * (MCH * c + kk + 1)]
                        nc.tensor.matmul(pl[:], lhsT=lhsT,
                                         rhs=s_h0T[:, BL * kk:BL * (kk + 1)],
                                         start=(kk == 0), stop=(kk == MCH - 1))
                    nc.scalar.activation(s_probsT[:, BL * c:BL * (c + 1)], pl[:],
                                         AF.Sigmoid)
                    ptile = pt_pool.tile([BL, 128], f32, tag="ptc")
                    nc.tensor.transpose(ptile[:],
                                        s_probsT[:, BL * c:BL * (c + 1)],
                                        s_ident[:])
                    nc.vector.tensor_copy(s_probs0[:, 128 * c:128 * (c + 1)],
                                          ptile[:])
                psl = slice(32 * n, 32 * (n + 1))
                nc.sync.dma_start(
                    out=s_p0g[psl, :],
                    in_=s_probs0[:, 512 * n:512 * (n + 1)]
                        .rearrange("b (q f) -> b q f", f=128),
                )
                nc.vector.tensor_copy(s_p0w[psl, :], s_p0g[psl, :])
                for r in range(ROUNDS):
                    sl = slice(8 * r, 8 * r + 8)
                    nc.vector.max(s_vals56[psl, sl], s_p0w[psl, :])
                    nc.vector.match_replace(s_p0w[psl, :], s_vals56[psl, sl],
                                            s_p0w[psl, :], -1.0)
            nc.sync.dma_start(out=out[:, 0:N0], in_=s_probs0[:])

            # ---- stage-2 merge: 16 sorted lists -> sorted top-56 per row ----
            nc.sync.dma_start(out=d_v56[:], in_=s_vals56[:])
            s_v896 = work.tile([BL, QG * NSEL], f32)
            nc.sync.dma_start(
                out=s_v896[:].rearrange("b (n q r) -> b n q r", n=NBLK, q=WPB),
                in_=d_v56[:].rearrange("(n b q) r -> b n q r", n=NBLK, b=BL),
            )
            s_vals1 = work.tile([BL, NSEL], f32)
            for r in range(ROUNDS):
                sl = slice(8 * r, 8 * r + 8)
                nc.vector.max(s_vals1[:, sl], s_v896[:])
                nc.vector.match_replace(s_v896[:], s_vals1[:, sl], s_v896[:], -1.0)

            s_gid56 = work.tile([BL, NSEL], u32)
            if not os.environ.get("KERNEL_GID_PE"):
                # baseline gid: scan the full [8, 2048] row per round
                for r in range(ROUNDS):
                    sl = slice(8 * r, 8 * r + 8)
                    nc.vector.max_index(s_gid56[:, sl], s_vals1[:, sl],
                                        s_probs0[:])
            else:
                # per-window max_index + PE selector combine
                pq = pg_pool.tile([128, NSEL], f32, tag="pq")
                nc.tensor.matmul(pq[:], lhsT=s_Gbc[:], rhs=s_vals1[:],
                                 start=True, stop=True)
                s_q56 = work.tile([128, NSEL], f32)
                nc.vector.tensor_copy(s_q56[:], pq[:])
                s_posu = work.tile([128, NSEL], u32)
                s_posf = work.tile([128, NSEL], f32)
                s_pm = work.tile([128, NSEL], f32)
                s_m = work.tile([128, NSEL], f32)
                pS = pg_pool.tile([BL, 2 * NSEL], f32, tag="pS")
                for r in range(ROUNDS):
                    sl = slice(8 * r, 8 * r + 8)
                    sl2 = slice(NSEL + 8 * r, NSEL + 8 * r + 8)
                    nc.vector.max_index(s_posu[:, sl], s_q56[:, sl], s_p0g[:])
                    nc.vector.tensor_copy(s_posf[:, sl], s_posu[:, sl])
                    # unmatched -> pos = 2^32-1 as float; mask = pos < 2048
                    nc.vector.tensor_scalar(s_m[:, sl], s_posf[:, sl], 2048.0,
                                            None, op0=ALU.is_lt)
                    nc.vector.tensor_mul(s_pm[:, sl], s_posf[:, sl], s_m[:, sl])
                    nc.tensor.matmul(pS[:, sl], lhsT=s_Gsel[:, 0:BL],
                                     rhs=s_pm[:, sl], start=True, stop=True)
                    nc.tensor.matmul(pS[:, sl2], lhsT=s_Gsel[:, BL:],
                                     rhs=s_m[:, sl], start=True, stop=True)
                # gid = S1 + 128*S2
                s_gidf = work.tile([BL, NSEL], f32)
                nc.vector.scalar_tensor_tensor(
                    out=s_gidf[:], in0=pS[:, NSEL:], scalar=128.0,
                    in1=pS[:, 0:NSEL], op0=ALU.mult, op1=ALU.add)
                nc.vector.tensor_copy(s_gid56[:], s_gidf[:])

            # ---- phase E: cand1 = clusters0[idx1] (DRAM-bounce reshapes) ----
            nc.sync.dma_start(out=d_idx1[:], in_=s_gid56[:, 0:K])
            s_idx1p = work.tile([100, 4], u32)
            nc.sync.dma_start(
                out=s_idx1p[:],
                in_=d_idx1[:].rearrange("(t P) one -> P (t one)", P=100),
            )
            s_c1raw = work.tile([100, 4 * CS], i32)
            for t in range(4):
                nc.gpsimd.indirect_dma_start(
                    out=s_c1raw[:, CS * t:CS * (t + 1)], out_offset=None,
                    in_=clusters0[:],
                    in_offset=bass.IndirectOffsetOnAxis(
                        ap=s_idx1p[:, t:t + 1], axis=0),
                )
            nc.sync.dma_start(
                out=d_cand1[:].rearrange("(t P m) one -> P t (m one)", P=100, m=CS),
                in_=s_c1raw[:],
            )
            s_cand1g = work.tile([128, NJ], i32)
            nc.sync.dma_start(
                out=s_cand1g[:],
                in_=d_cand1[:].rearrange("(b q j) one -> (b q) (j one)", q=QG, j=NJ),
            )

            # ---- phase F: gather C1 rows + fused dots ----
            s_logits1g = work.tile([128, NJ], f32)
            s_scr = work.tile([128, EMB], f32)
            for ch in range(NCHUNK):
                et = ev_pool.tile([128, JPC * EMB], f32, tag="ev")
                for jj in range(JPC):
                    j = JPC * ch + jj
                    nc.gpsimd.indirect_dma_start(
                        out=et[:, EMB * jj:EMB * (jj + 1)], out_offset=None,
                        in_=C1[:],
                        in_offset=bass.IndirectOffsetOnAxis(
                            ap=s_cand1g[:, j:j + 1], axis=0),
                    )
                for jj in range(JPC):
                    j = JPC * ch + jj
                    nc.vector.scalar_tensor_tensor(
                        out=s_scr[:], in0=et[:, EMB * jj:EMB * (jj + 1)],
                        scalar=1.0, in1=s_f1rep[:],
                        op0=ALU.mult, op1=ALU.mult,
                        accum_out=s_logits1g[:, j:j + 1])

            # ---- phase G: probs1, top-50, w1 ----
            s_probs1g = work.tile([128, NJ], f32)
            nc.scalar.activation(s_probs1g[:], s_logits1g[:], AF.Sigmoid)
            s_probs1b = work.tile([BL, NCAND], f32)
            nc.sync.dma_start(
                out=s_probs1b[:].rearrange("b (q j) -> b q j", j=NJ),
                in_=s_probs1g[:],
            )
            s_p1w = work.tile([BL, NCAND], f32)
            nc.vector.tensor_copy(s_p1w[:], s_probs1b[:])
            s_vals2 = work.tile([BL, NSEL], f32)
            s_pos2 = work.tile([BL, NSEL], u32)
            for r in range(ROUNDS):
                sl = slice(8 * r, 8 * r + 8)
                nc.vector.max(s_vals2[:, sl], s_p1w[:])
                nc.vector.max_index(s_pos2[:, sl], s_vals2[:, sl], s_p1w[:])
                nc.vector.match_replace(s_p1w[:], s_vals2[:, sl], s_p1w[:], -1.0)
            s_g1 = work.tile([BL, NCAND], f32)
            nc.vector.tensor_copy(
                s_g1[:].rearrange("b (k m) -> b k m", m=CS),
                s_vals1[:, 0:K].to_broadcast([BL, K, CS]),
            )
            s_w1 = work.tile([BL, NCAND], f32)
            nc.vector.tensor_mul(s_w1[:], s_probs1b[:], s_g1[:])
            nc.sync.dma_start(out=out[:, N0:N0 + NCAND], in_=s_w1[:])

            # ---- phase H: level-2 candidate ids ----
            s_fidx = work.tile([BL, K], u32)
            nc.vector.tensor_tensor(s_fidx[:], s_pos2[:, 0:K],
                                    s_brow400[:].to_broadcast([BL, K]),
                                    op=ALU.add)
            nc.sync.dma_start(out=d_fidx[:], in_=s_fidx[:])
            s_fidxp = work.tile([100, 4], u32)
            nc.sync.dma_start(
                out=s_fidxp[:],
                in_=d_fidx[:].rearrange("(t P) one -> P (t one)", P=100),
            )
            s_ind2raw = work.tile([100, 4], i32)
            for t in range(4):
                nc.gpsimd.indirect_dma_start(
                    out=s_ind2raw[:, t:t + 1], out_offset=None, in_=d_cand1[:],
                    in_offset=bass.IndirectOffsetOnAxis(
                        ap=s_fidxp[:, t:t + 1], axis=0),
                )
            s_c2raw = work.tile([100, 4 * CS], i32)
            for t in range(4):
                nc.gpsimd.indirect_dma_start(
                    out=s_c2raw[:, CS * t:CS * (t + 1)], out_offset=None,
                    in_=clusters1[:],
                    in_offset=bass.IndirectOffsetOnAxis(
                        ap=s_ind2raw[:, t:t + 1], axis=0),
                )
            nc.sync.dma_start(
                out=d_cand2[:].rearrange("(t P m) one -> P t (m one)", P=100, m=CS),
                in_=s_c2raw[:],
            )
            s_cand2g = work.tile([128, NJ], i32)
            nc.sync.dma_start(
                out=s_cand2g[:],
                in_=d_cand2[:].rearrange("(b q j) one -> (b q) (j one)", q=QG, j=NJ),
            )

            # ---- phase I: gather C2 rows + fused dots, probs2, w2 ----
            s_logits2g = work.tile([128, NJ], f32)
            for ch in range(NCHUNK):
                et = ev_pool.tile([128, JPC * EMB], f32, tag="ev")
                for jj in range(JPC):
                    j = JPC * ch + jj
                    nc.gpsimd.indirect_dma_start(
                        out=et[:, EMB * jj:EMB * (jj + 1)], out_offset=None,
                        in_=C2[:],
                        in_offset=bass.IndirectOffsetOnAxis(
                            ap=s_cand2g[:, j:j + 1], axis=0),
                    )
                for jj in range(JPC):
                    j = JPC * ch + jj
                    nc.vector.scalar_tensor_tensor(
                        out=s_scr[:], in0=et[:, EMB * jj:EMB * (jj + 1)],
                        scalar=1.0, in1=s_f2rep[:],
                        op0=ALU.mult, op1=ALU.mult,
                        accum_out=s_logits2g[:, j:j + 1])
            s_probs2g = work.tile([128, NJ], f32)
            nc.scalar.activation(s_probs2g[:], s_logits2g[:], AF.Sigmoid)
            s_mask = work.tile([128, NJ], f32)
            nc.vector.tensor_scalar(s_mask[:], s_logits2g[:], 0.0, None,
                                    op0=ALU.not_equal)
            nc.vector.tensor_mul(s_probs2g[:], s_probs2g[:], s_mask[:])
            s_probs2b = work.tile([BL, NCAND], f32)
            nc.sync.dma_start(
                out=s_probs2b[:].rearrange("b (q j) -> b q j", j=NJ),
                in_=s_probs2g[:],
            )
            s_g2 = work.tile([BL, NCAND], f32)
            nc.vector.tensor_copy(
                s_g2[:].rearrange("b (k m) -> b k m", m=CS),
                s_vals2[:, 0:K].to_broadcast([BL, K, CS]),
            )
            s_w2 = work.tile([BL, NCAND], f32)
            nc.vector.tensor_mul(s_w2[:], s_probs2b[:], s_g2[:])
            nc.sync.dma_start(out=out[:, N0 + NCAND:OUTW], in_=s_w2[:])

    nc.compile()
    return nc


def _get_nc():
    if "nc" not in _cached:
        _cached["nc"] = _build()
    return _cached["nc"]


def _make_in_maps(feat0, feat1, feat2, Wh, bh, C0, b0, C1, b1, C2, b2,
                  clusters0, clusters1):
    WhT = np.ascontiguousarray(Wh.T)            # [1536, 768]
    C0T = np.ascontiguousarray(C0.T)            # [768, 2048]
    feat0T = np.ascontiguousarray(feat0.T)      # [1536, 64]
    WhP = np.ascontiguousarray(
        WhT.reshape(KCH0, 128, EMB).transpose(1, 0, 2).reshape(128, KCH0 * EMB))
    # lhsT chunks: C0P[:, 128*(6c+kk):...] = C0T[128kk:128kk+128, 128c:128c+128]
    C0P = np.ascontiguousarray(
        C0T.reshape(MCH, 128, 16, 128).transpose(1, 2, 0, 3)
           .reshape(128, 16 * MCH * 128))
    brow400 = (NCAND * np.arange(BL, dtype=np.uint32)).reshape(BL, 1)
    c0 = np.ascontiguousarray(clusters0.astype(np.int32))
    c1 = np.ascontiguousarray(clusters1.astype(np.int32))
    # level-0 g-layout: p = 32n + 4b + q_l; row(p) = (p%32)//4,
    # window(p) = 4*(p//32) + p%4
    p = np.arange(128)
    rowp = (p % 32) // 4
    winp = 4 * (p // 32) + p % 4
    gb = (rowp[:, None] == np.arange(BL)[None, :]).astype(np.float32)
    gq = gb * winp[:, None].astype(np.float32)
    Gsel_np = np.ascontiguousarray(np.concatenate([gb, gq], axis=1))
    Gbc_np = np.ascontiguousarray(gb.T)
    in_maps = []
    for c in range(NCORES):
        rows = slice(BL * c, BL * (c + 1))
        f0P = np.ascontiguousarray(
            feat0T[:, rows].reshape(KCH0, 128, BL).transpose(1, 0, 2)
                  .reshape(128, KCH0 * BL))
        in_maps.append({
            "feat0P": f0P,
            "WhP": WhP,
            "C0P": C0P,
            "f1rep": np.repeat(feat1[rows], QG, axis=0),
            "f2rep": np.repeat(feat2[rows], QG, axis=0),
            "C1": C1,
            "C2": C2,
            "clusters0": c0,
            "clusters1": c1,
            "Gsel": Gsel_np,
            "Gbc": Gbc_np,
            "brow400": brow400,
        })
    return in_maps


def kernel(**inputs):
    nc = _get_nc()
    in_maps = _make_in_maps(**inputs)
    if os.environ.get("BASS_KERNEL_SIM"):
        from concourse.bass_interp import CoreSim
        outs = []
        for c in range(NCORES):
            sim = CoreSim(nc)
            for name, arr in in_maps[c].items():
                sim.tensor(name)[:] = arr
            sim.simulate()
            outs.append(np.array(sim.tensor("out")))
        return np.concatenate(outs, axis=0)
    from concourse.bass_utils import run_bass_kernel_spmd
    trace = bool(os.environ.get("BASS_KERNEL_TRACE"))
    res = run_bass_kernel_spmd(nc, in_maps, core_ids=list(range(NCORES)),
                               trace=trace)
    _cached["last_exec_ns"] = res.exec_time_ns
    _cached["last_results"] = res
    return np.concatenate([res.results[c]["out"] for c in range(NCORES)], axis=0)


if __name__ == "__main__":
    _get_nc()
    print("build+compile OK")
